# revision 1
# baseline (speedup 1.0000x reference)
"""ListFoldLoss Trainium2 kernel (8-core SPMD, Bass/Tile).

Math: the reference builds D[u,v] = exp(f_u - f_v) (rank-1: exp(f) x exp(-f))
and takes nested-window sums of it.  Every window sum factorizes:

    S(i) = A(i) * B(i),   A(i) = sum_{rank in [i, N-i)} exp(pred),
                          B(i) = sum_{rank in [i, N-i)} exp(-pred)

so the NxN matrix never needs to exist.  With r_u = rank of element u when
sorting by target descending and m_u = min(r_u, N-1-r_u):

    A(i)  = sum_u exp(pred_u)  * [m_u >= i]
    B(i)  = sum_u exp(-pred_u) * [m_u >= i]
    num   = sum_u pred_u * (2*[r_u < N/2] - 1)        (= sum_i log num_i)
    loss  = sum_i log(A(i)*B(i) - (N-2i)) - num

Two-level window sums: write i = 32*Q + S (Q in [0,128), S in [0,32)) and
q_u = floor(m_u/32), s_u = m_u mod 32.  Then [m >= i] = [q > Q] + [q==Q][s>=S]:

    A(32Q+S) = SufH_a[Q] + F_a[Q,S]
    H_a[Q]   = sum_u a_u [q_u == Q]          (coarse histogram)
    SufH_a[Q]= sum_{Q' > Q} H_a[Q']          (strict suffix, one tri-matmul)
    F_a[Q,S] = sum_u a_u [q_u == Q][s_u >= S]

F and SufH are built from per-core partial histograms over each core's own
1024 elements (tiny per-subtile masks + PE matmuls into PSUM; SufH comes
from the shifted step mask [q_u > Q] directly) and combined with a single
ReduceScatter of [128, 67] f32 -> [16, 67] per core (64 F cols | 2 SufH
cols | num partial, which lands on core 0 only).  Each core then computes
log-denominators for its own 512 windows; the host sums the 8 partials.

Device plan (per core c of 8):
  phase 1: ranks r_u for u in the core's 1024-slice, via N comparisons per
           u against a bf16 broadcast of target (rank errors from bf16 ties
           perturb the loss ~3e-4 rel, well inside the 2e-2 gate):
           DVE fused tensor_scalar(is_gt)+accum (4x mode) for 24 of 32
           (subtile, chunk) units, ACT fused sign(t_j - t_u)+accum for 8.
  phase 1.5: m = min(r, N-1-r); step masks [m >= 32Q'] (q+1 via accum);
           local num partial.
  phase 2: per u-subtile masks OQ (step diff) and SS=[s>=S], weighted SSab,
           PE-accumulated into F PSUM [128, 64] and SufH PSUM [128, 2].
  collective: ReduceScatter [128, 67] -> [16, 67].
  phase 3: denom/ln over the core's 512 windows, minus num on core 0.

`reps` replicates the whole body serially inside one NEFF -- used only for
slope-based wall-clock timing (NTFF profiling is unavailable here).
"""

import numpy as np

import concourse.bacc as bacc
import concourse.bass as bass
import concourse.mybir as mybir
import concourse.tile as tile

N = 8192
NCORE = 8
P = 128
US = N // NCORE          # 1024 u's per core
UT = US // P             # 8 u-subtiles per core
NPAIR = N // 2           # 4096 loss terms
NQ = 128                 # coarse window blocks (i = 32Q + S)
NS = 32
NCHUNK = 4               # j-dim chunks (2048 each); (k, q) units split 3-way
# phase-1 engine split in (k, q) unit order: first ND units on DVE (bf16
# compares, 4x mode), rest on ACT (sign trick, f32 rate).  24/8 balances the
# model rates 3.84 : 1.2 elem/ns.  (Pool cannot run tensor_scalar.)
ND_UNITS = 25
NA_UNITS = 7

F32 = mybir.dt.float32
BF16 = mybir.dt.bfloat16
I16 = mybir.dt.int16
AF = mybir.ActivationFunctionType
OP = mybir.AluOpType


CW = N // NCHUNK
UNITS = [(k, q) for k in range(UT) for q in range(NCHUNK)]


def _unit_engines(nd_units: int, na_units: int) -> dict:
    return {
        u: ("d" if i < nd_units else ("a" if i < nd_units + na_units else "p"))
        for i, u in enumerate(UNITS)
    }


def build_module(
    debug: bool = False,
    reps: int = 1,
    collective: bool = True,
    nd_units: int = ND_UNITS,
    na_units: int = NA_UNITS,
    work_bufs: int = 2,
):
    nch = NCHUNK
    bounds = tuple(q * CW for q in range(nch + 1))
    eng = _unit_engines(nd_units, na_units)
    cols = {"d": {}, "a": {}, "p": {}}
    for u in UNITS:
        cols[eng[u]][u] = len(cols[eng[u]])
    nc = bacc.Bacc(
        "TRN2",
        target_bir_lowering=False,
        debug=False,
        enable_asserts=False,
        num_devices=NCORE,
    )

    t_row = nc.dram_tensor("t_row", [1, N], BF16, kind="ExternalInput")
    # packed small consts: [tcol 8 | pcol 8 | corr 8 | win 32] per partition
    NPK = 3 * UT + NS
    packed = nc.dram_tensor("packed", [P, NPK], F32, kind="ExternalInput")
    out_part = nc.dram_tensor("out_part", [1, 1], F32, kind="ExternalOutput")
    if debug:
        dbg_r = nc.dram_tensor("dbg_r", [P, UT], F32, kind="ExternalOutput")
        dbg_m = nc.dram_tensor("dbg_m", [P, UT], F32, kind="ExternalOutput")
        dbg_fh = nc.dram_tensor("dbg_fh", [P // NCORE, 67], F32, kind="ExternalOutput")

    with tile.TileContext(nc) as tc:
        with (
            tc.tile_pool(name="consts", bufs=1) as consts,
            tc.tile_pool(name="rep", bufs=2) as rp,
            tc.tile_pool(name="work", bufs=work_bufs) as work,
            tc.tile_pool(name="psum", bufs=2, space="PSUM") as psum,
            tc.tile_pool(name="dram", bufs=2, space="DRAM") as dram,
        ):
            # ---- constant/small loads ----
            # packed leads the sync ring so tcol/negt are ready before the
            # first tb chunk lands (the ACT ring is busy with table loads)
            packed_sb = consts.tile([P, NPK], F32)
            nc.sync.dma_start(packed_sb[:], packed.ap())
            tcol_sb = packed_sb[:, 0:UT]
            pcol_sb = packed_sb[:, UT : 2 * UT]
            ccol_sb = packed_sb[:, 2 * UT : 3 * UT]

            # on-device iotas (Pool engine, init-time only)
            is_i = consts.tile([P, NS], I16)
            nc.gpsimd.iota(is_i[:], [[1, NS]], base=0, channel_multiplier=0)

            iotaS = consts.tile([P, NS], F32)
            nc.vector.tensor_copy(iotaS[:], is_i[:])
            # 32-grid thresholds 0,32,...,4096 and offset iota -32..-1
            i32_i = consts.tile([P, NQ + 1], I16)
            nc.gpsimd.iota(i32_i[:], [[32, NQ + 1]], base=0, channel_multiplier=0)
            iota32e = consts.tile([P, NQ + 1], F32)
            nc.vector.tensor_copy(iota32e[:], i32_i[:])
            iotaSm = consts.tile([P, NS], F32)
            nc.vector.tensor_scalar(iotaSm[:], iotaS[:], -32.0, None, OP.add)
            win_sb = packed_sb[:, 3 * UT : 3 * UT + NS]

            negt = consts.tile([P, UT], F32)
            nc.vector.tensor_scalar(negt[:], tcol_sb, -1.0, None, OP.mult)
            ones_col = consts.tile([P, 1], F32)
            nc.vector.memset(ones_col[:], 1.0)

            # own-slice weights: a = exp(p), b = exp(-p), f32 + bf16
            ab = consts.tile([P, UT, 2], F32)
            nc.scalar.activation(ab[:, :, 0], pcol_sb, AF.Exp)
            nc.scalar.activation(ab[:, :, 1], pcol_sb, AF.Exp, scale=-1.0)
            ab_bf = consts.tile([P, UT, 2], BF16)
            nc.vector.tensor_copy(ab_bf[:], ab[:])

            for _rep in range(reps):
                # ---- phase 1: ranks, (k, q) units split over 3 engines ----
                racc_d = rp.tile([P, max(len(cols["d"]), 1)], F32, tag="racc_d")
                racc_a = rp.tile([P, max(len(cols["a"]), 1)], F32, tag="racc_a")
                racc_p = rp.tile([P, max(len(cols["p"]), 1)], F32, tag="racc_p")
                racc = {"d": racc_d, "a": racc_a, "p": racc_p}
                # keep the gpsimd ring free for the collective and the ACT
                # ring free for compute: all broadcast chunks ride sync
                dma_rings = (nc.sync, nc.sync, nc.sync, nc.sync)
                for q in range(nch):
                    lo, hi = bounds[q], bounds[q + 1]
                    tb = work.tile([P, CW], BF16, tag=f"tb{q}")
                    dma_rings[q % len(dma_rings)].dma_start(
                        tb[:],
                        t_row.ap()[0:1, lo:hi].to_broadcast((P, CW)),
                    )
                    scr_d = rp.tile([P, CW], BF16, tag="scr_d")
                    scr_a = rp.tile([P, CW], F32, tag="scr_a")
                    for k in range(UT):
                        e = eng[(k, q)]
                        ci = cols[e][(k, q)]
                        col = racc[e][:, ci : ci + 1]
                        if e == "d":
                            nc.vector.tensor_scalar(
                                scr_d[:], tb[:], tcol_sb[:, k : k + 1], None,
                                OP.is_gt, OP.add, accum_out=col,
                            )
                        else:
                            nc.scalar.activation(
                                scr_a[:], tb[:], AF.Sign,
                                bias=negt[:, k : k + 1], accum_out=col,
                            )

                # combine per-unit sums into ranks:
                #   r_k = cntD + cntP + (signA + corr_k) / 2
                # corr_k = width(A windows) - [own j in an A window] (input col)
                rsum = rp.tile([P, UT], F32, tag="rsum")
                tmp = rp.tile([P, UT], F32, tag="tmp")
                m_col = rp.tile([P, UT], F32, tag="m_col")
                step = rp.tile([P, UT, NQ + 1], BF16, tag="step")
                qp1 = rp.tile([P, UT], F32, tag="qp1")
                sm_col = rp.tile([P, UT], F32, tag="sm_col")
                f_ps = psum.tile([P, 64], F32, tag="f_ps")
                h_ps = psum.tile([P, 2], F32, tag="h_ps")

                def emit_half(ka3, kb3):
                    # m, step masks, and F/SufH matmuls for subtiles [ka3, kb3)
                    sl = slice(ka3, kb3)
                    nc.vector.tensor_scalar(
                        tmp[:, sl], rsum[:, sl], float(N - 1), -1.0,
                        OP.subtract, OP.mult,
                    )
                    nc.vector.tensor_tensor(
                        m_col[:, sl], rsum[:, sl], tmp[:, sl], OP.min
                    )
                    for k in range(ka3, kb3):
                        nc.vector.tensor_scalar(
                            step[:, k, :], iota32e[:], m_col[:, k : k + 1], None,
                            OP.is_le, OP.add, accum_out=qp1[:, k : k + 1],
                        )
                    # s - 32 = m - 32*(q+1), compared against iotaSm (-32..-1)
                    nc.vector.scalar_tensor_tensor(
                        sm_col[:, sl], qp1[:, sl], -32.0, m_col[:, sl],
                        OP.mult, OP.add,
                    )
                    for k in range(ka3, kb3):
                        oq = work.tile([P, NQ], BF16, tag="oq")
                        nc.vector.tensor_tensor(
                            oq[:], step[:, k, 0:NQ], step[:, k, 1 : NQ + 1],
                            OP.subtract,
                        )
                        # fused: ([S-32 <= s-32] mult a) in one two-scalar op
                        ssab = work.tile([P, 2 * NS], BF16, tag="ssab")
                        nc.vector.tensor_scalar(
                            ssab[:, 0:NS], iotaSm[:], sm_col[:, k : k + 1],
                            ab[:, k, 0:1], OP.is_le, OP.mult,
                        )
                        nc.vector.tensor_scalar(
                            ssab[:, NS : 2 * NS], iotaSm[:], sm_col[:, k : k + 1],
                            ab[:, k, 1:2], OP.is_le, OP.mult,
                        )
                        nc.tensor.matmul(
                            f_ps[:], lhsT=oq[:], rhs=ssab[:],
                            start=(k == 0), stop=(k == UT - 1),
                        )
                        # SufH[Q] = sum_u ab_u [q_u > Q] via the shifted step
                        nc.tensor.matmul(
                            h_ps[:], lhsT=step[:, k, 1 : NQ + 1],
                            rhs=ab_bf[:, k, :],
                            start=(k == 0), stop=(k == UT - 1),
                        )

                fd = nd_units // nch  # leading fully-DVE subtiles
                if fd:
                    nc.vector.tensor_reduce(
                        rsum[:, 0:fd],
                        racc_d[:, 0 : fd * nch].rearrange(
                            "p (k q) -> p k q", q=nch
                        ),
                        axis=mybir.AxisListType.X,
                        op=OP.add,
                    )
                    # this half only depends on DVE compares: it pipelines
                    # under the ACT sign tail and the k6+ combine below
                    emit_half(0, fd)
                for k in range(fd, UT):
                    kd = [cols["d"][(k, q)] for q in range(nch) if eng[(k, q)] == "d"]
                    ka_ = [cols["a"][(k, q)] for q in range(nch) if eng[(k, q)] == "a"]
                    kp = [cols["p"][(k, q)] for q in range(nch) if eng[(k, q)] == "p"]
                    terms = []
                    if ka_:
                        if len(ka_) > 1:
                            sa = rp.tile([P, 1], F32, tag=f"sa{k}")
                            nc.vector.tensor_reduce(
                                sa[:], racc_a[:, ka_[0] : ka_[-1] + 1],
                                axis=mybir.AxisListType.X, op=OP.add,
                            )
                            sav = sa[:]
                        else:
                            sav = racc_a[:, ka_[0] : ka_[0] + 1]
                        tk = rp.tile([P, 1], F32, tag=f"tk{k}")
                        nc.vector.tensor_scalar(
                            tk[:], sav, ccol_sb[:, k : k + 1], 0.5,
                            OP.add, OP.mult,
                        )
                        terms.append(tk[:])
                    for tile_, rng, nm in ((racc_d, kd, "d"), (racc_p, kp, "p")):
                        if not rng:
                            continue
                        if len(rng) > 1:
                            rd = rp.tile([P, 1], F32, tag=f"rd{nm}{k}")
                            nc.vector.tensor_reduce(
                                rd[:], tile_[:, rng[0] : rng[-1] + 1],
                                axis=mybir.AxisListType.X, op=OP.add,
                            )
                            terms.append(rd[:])
                        else:
                            terms.append(tile_[:, rng[0] : rng[0] + 1])
                    tgt = rsum[:, k : k + 1]
                    if len(terms) == 1:
                        nc.vector.tensor_copy(tgt, terms[0])
                    else:
                        nc.vector.tensor_tensor(tgt, terms[0], terms[1], OP.add)
                        for t2 in terms[2:]:
                            nc.vector.tensor_tensor(tgt, tgt, t2, OP.add)
                if debug:
                    nc.sync.dma_start(dbg_r.ap(), rsum[:])

                # second half: subtiles with ACT/Pool contributions
                emit_half(fd, UT)
                if debug:
                    nc.sync.dma_start(dbg_m.ap(), m_col[:])

                sgn = rp.tile([P, UT], F32, tag="sgn")
                nc.vector.tensor_scalar(sgn[:], rsum[:], float(NPAIR), None, OP.is_lt)
                nc.vector.tensor_scalar(sgn[:], sgn[:], 2.0, -1.0, OP.mult, OP.add)
                xp = rp.tile([P, UT], F32, tag="xp")
                nc.vector.tensor_tensor(xp[:], sgn[:], pcol_sb, OP.mult)
                xq = rp.tile([P, 1], F32, tag="xq")
                nc.vector.tensor_reduce(
                    xq[:], xp[:], axis=mybir.AxisListType.X, op=OP.add
                )
                np_ps = psum.tile([1, 1], F32, tag="np_ps")
                nc.tensor.matmul(
                    np_ps[:], lhsT=xq[:], rhs=ones_col[:], start=True, stop=True
                )

                fh_in = rp.tile([P, 67], F32, tag="fh_in")
                nc.vector.tensor_copy(fh_in[:, 0:64], f_ps[:])
                nc.vector.tensor_copy(fh_in[:, 64:66], h_ps[:])
                nc.vector.memset(fh_in[:, 66:67], 0.0)
                nc.vector.tensor_copy(fh_in[0:1, 66:67], np_ps[0:1, :])

                # ReduceScatter: core c receives the summed Q-rows
                # [16c, 16c+16) on partitions 0..15 (1.0x cost vs 1.875x AR)
                QC = P // NCORE  # 16 Q rows per core
                fh_dram = dram.tile([P, 67], F32, tag="fh_dram")
                nc.sync.dma_start(fh_dram[:], fh_in[:])
                fhrs_dram = dram.tile([QC, 67], F32, tag="fhrs_dram")
                if collective:
                    nc.gpsimd.collective_compute(
                        "ReduceScatter",
                        OP.add,
                        replica_groups=[list(range(NCORE))],
                        ins=[fh_dram[:].opt()],
                        outs=[fhrs_dram[:].opt()],
                    )
                else:  # timing-sim variant: stand-in DMA, wrong data, same shapes
                    nc.sync.dma_start(fhrs_dram[:], fh_dram[0:QC, :])
                fh_sb = rp.tile([QC, 67], F32, tag="fh_sb")
                nc.sync.dma_start(fh_sb[:], fhrs_dram[:])
                if debug:
                    nc.sync.dma_start(dbg_fh.ap(), fh_sb[:])

                # ---- phase 3: denom/ln on this core's 512 windows ----
                at = rp.tile([QC, NS], F32, tag="at")
                nc.vector.tensor_scalar(
                    at[:], fh_sb[:, 0:NS], fh_sb[:, 64:65], None, OP.add
                )
                bt = rp.tile([QC, NS], F32, tag="bt")
                nc.vector.tensor_scalar(
                    bt[:], fh_sb[:, NS : 2 * NS], fh_sb[:, 65:66], None, OP.add
                )
                den = rp.tile([QC, NS], F32, tag="den")
                nc.vector.tensor_tensor(den[:], at[:], bt[:], OP.mult)
                nc.vector.tensor_tensor(den[:], den[:], win_sb[0:QC, :], OP.subtract)
                # NOT dead code: bf16 rank ties can leave the innermost
                # window empty (A*B = 0, den = -2); this clamp mirrors the
                # reference's where(denom <= 0, EPS, denom) guard
                nc.vector.tensor_scalar(den[:], den[:], 1e-8, None, OP.max)
                logd = rp.tile([QC, NS], F32, tag="logd")
                lnacc = rp.tile([QC, 1], F32, tag="lnacc")
                nc.scalar.activation(logd[:], den[:], AF.Ln, accum_out=lnacc[:])
                ln_ps = psum.tile([1, 1], F32, tag="ln_ps")
                nc.tensor.matmul(
                    ln_ps[:], lhsT=lnacc[:], rhs=ones_col[0:QC, :],
                    start=True, stop=True,
                )
                out_sb = rp.tile([1, 1], F32, tag="out_sb")
                nc.vector.tensor_tensor(
                    out_sb[:], ln_ps[0:1, :], fh_sb[0:1, 66:67], OP.subtract
                )
                nc.sync.dma_start(out_part.ap(), out_sb[:])

    nc.compile()
    return nc


def make_in_maps(pred: np.ndarray, target: np.ndarray):
    import ml_dtypes

    pred = np.ascontiguousarray(pred, dtype=np.float32).reshape(N)
    target = np.ascontiguousarray(target, dtype=np.float32).reshape(N)
    t_row = np.ascontiguousarray(target.reshape(1, N).astype(ml_dtypes.bfloat16))
    eng = _unit_engines(ND_UNITS, NA_UNITS)
    in_maps = []
    for c in range(NCORE):
        tsl = target[c * US : (c + 1) * US]
        psl = pred[c * US : (c + 1) * US]
        own_chunk = (c * US) // CW
        corr = np.zeros(UT, np.float32)
        for k in range(UT):
            w_a = CW * sum(1 for q in range(NCHUNK) if eng[(k, q)] == "a")
            eq = 1.0 if eng[(k, own_chunk)] == "a" else 0.0
            corr[k] = w_a - eq
        # win[rho, S] = N - 2*(32*(16c + rho) + S*... ) on rows 0..15
        win = np.zeros((P, NS), np.float32)
        rho = np.arange(P // NCORE)
        s_i = np.arange(NS)
        win[: P // NCORE, :] = (
            N - 64.0 * (16 * c + rho)[:, None] - 2.0 * s_i[None, :]
        )
        pk = np.concatenate(
            [
                tsl.reshape(UT, P).T,
                psl.reshape(UT, P).T,
                np.broadcast_to(corr, (P, UT)),
                win,
            ],
            axis=1,
        ).astype(np.float32)
        in_maps.append(
            {
                "t_row": t_row,
                "packed": np.ascontiguousarray(pk),
            }
        )
    return in_maps


_CACHE = {}


def _get_module():
    if "nc" not in _CACHE:
        _CACHE["nc"] = build_module(debug=False)
    return _CACHE["nc"]


def kernel(pred: np.ndarray, target: np.ndarray) -> np.ndarray:
    from concourse import bass_utils

    nc = _get_module()
    in_maps = make_in_maps(pred, target)
    res = bass_utils.run_bass_kernel_spmd(nc, in_maps, core_ids=list(range(NCORE)))
    total = np.float32(0.0)
    for c in range(NCORE):
        total = np.float32(total + res.results[c]["out_part"][0, 0])
    return np.asarray(total, dtype=np.float32)



# revision 2
# speedup vs baseline: 2.3936x; 2.3936x over previous
"""ListFoldLoss Trainium2 kernel v1 (8-core SPMD, Bass/Tile).

Same math as the baseline (rank-1 factorization of the psi matrix; see
kernel.py docstring) but phase 1 (ranks) is computed via a quantized-key
histogram instead of N brute-force compares per element:

  key_u = round(t_u * SCALE + OFF) as int16 in [0, 16384)   (14 bits)
  hi = round((key - 63.5)/128)  (= floor(key/128)),  lo = key mod 128
  r_u = #{j: key_j > key_u} = SufCntHi[hi_u] + SufG_lo[hi_u, lo_u]

Per core: one-hot the own 1024 keys' digits (is_equal vs iota), PE-matmul
them into a local 2D count histogram G_T[l, h]; AllReduce G (64KB); build
  HiCnt[h]    = sum_l G_T[l, h]                    (PE: G_T^T @ ones)
  SufRow[h]   = sum_{h'>h} HiCnt[h']               (PE: HC^T @ STRI)
  rhs2[l', h] = sum_{l>l'} G_T[l, h] + SufRow[h]   (PE: STRI^T@G_T + outer)
then per u-subtile W2 = OHlu^T @ rhs2 gives row u = SufG_lo[:, lo_u] +
SufRow[:], and a tensor_tensor_reduce pick against the hi one-hot yields
r_u exactly.  Quantization ties (elements sharing a 5.4e-4-wide key bin get
equal ranks) perturb the loss like the baseline's bf16 ties (~1e-4 rel).

Phases 1.5/2/3 are the baseline's: m = min(r, N-1-r), 32-grid step masks,
F/SufH PSUM matmuls, ReduceScatter [128,67] -> [16,67], per-core denom/ln,
host-summed partials.
"""

import numpy as np

import concourse.bacc as bacc
import concourse.bass as bass
import concourse.mybir as mybir
import concourse.tile as tile

N = 8192
NCORE = 8
P = 128
US = N // NCORE          # 1024 u's per core
UT = US // P             # 8 u-subtiles per core
NPAIR = N // 2           # 4096 loss terms
NQ = 128                 # coarse window blocks (i = 32Q + S)
NS = 32

SCALE = 1836.0
OFF = 8192.0

F32 = mybir.dt.float32
BF16 = mybir.dt.bfloat16
I16 = mybir.dt.int16
AF = mybir.ActivationFunctionType
OP = mybir.AluOpType


def build_module(
    debug: bool = False,
    reps: int = 1,
    collective: bool = True,
    work_bufs: int = 2,
    coll1: bool | None = None,
    coll2: bool | None = None,
):
    coll1 = collective if coll1 is None else coll1
    coll2 = collective if coll2 is None else coll2
    nc = bacc.Bacc(
        "TRN2",
        target_bir_lowering=False,
        debug=False,
        enable_asserts=False,
        num_devices=NCORE,
    )

    t_own = nc.dram_tensor("t_own", [1, US], F32, kind="ExternalInput")
    # packed small consts: [tcol 8 | pcol 8 | win 32] per partition
    NPK = 2 * UT + NS
    packed = nc.dram_tensor("packed", [P, NPK], F32, kind="ExternalInput")
    out_part = nc.dram_tensor("out_part", [1, 1], F32, kind="ExternalOutput")
    if debug:
        dbg_r = nc.dram_tensor("dbg_r", [P, UT], F32, kind="ExternalOutput")
        dbg_m = nc.dram_tensor("dbg_m", [P, UT], F32, kind="ExternalOutput")
        dbg_g = nc.dram_tensor("dbg_g", [P, P], F32, kind="ExternalOutput")
        dbg_fh = nc.dram_tensor("dbg_fh", [P // NCORE, 67], F32, kind="ExternalOutput")

    with tile.TileContext(nc) as tc:
        with (
            tc.tile_pool(name="consts", bufs=1) as consts,
            tc.tile_pool(name="rep", bufs=2) as rp,
            tc.tile_pool(name="work", bufs=work_bufs) as work,
            tc.tile_pool(name="psA", bufs=1, space="PSUM") as psA,
            tc.tile_pool(name="psB", bufs=2, space="PSUM") as psB,
            tc.tile_pool(name="dram", bufs=2, space="DRAM") as dram,
        ):
            # ---- constant/small loads ----
            packed_sb = consts.tile([P, NPK], F32)
            nc.sync.dma_start(packed_sb[:], packed.ap())
            tcol_sb = packed_sb[:, 0:UT]
            pcol_sb = packed_sb[:, UT : 2 * UT]
            win_sb = packed_sb[:, 2 * UT : 2 * UT + NS]

            # on-device iotas / masks (init-time only)
            iota_row_i = consts.tile([P, P], I16)
            nc.gpsimd.iota(iota_row_i[:], [[1, P]], base=0, channel_multiplier=0)
            iota_col_i = consts.tile([P, 1], I16)
            nc.gpsimd.iota(iota_col_i[:], [[0, 1]], base=0, channel_multiplier=1)
            iota_row_f = consts.tile([P, P], F32)
            nc.vector.tensor_copy(iota_row_f[:], iota_row_i[:])
            iota_col_f = consts.tile([P, 1], F32)
            nc.vector.tensor_copy(iota_col_f[:], iota_col_i[:])
            # STRI[p, f] = [p > f]  (strict lower in (p, f)); f32 for fp32 matmuls
            stri = consts.tile([P, P], F32)
            nc.vector.tensor_scalar(
                stri[:], iota_row_f[:], iota_col_f[:], None, OP.is_lt
            )

            is_i = consts.tile([P, NS], I16)
            nc.gpsimd.iota(is_i[:], [[1, NS]], base=0, channel_multiplier=0)
            iotaS = consts.tile([P, NS], F32)
            nc.vector.tensor_copy(iotaS[:], is_i[:])
            i32_i = consts.tile([P, NQ + 1], I16)
            nc.gpsimd.iota(i32_i[:], [[32, NQ + 1]], base=0, channel_multiplier=0)
            iota32e = consts.tile([P, NQ + 1], F32)
            nc.vector.tensor_copy(iota32e[:], i32_i[:])
            iotaSm = consts.tile([P, NS], F32)
            nc.vector.tensor_scalar(iotaSm[:], iotaS[:], -32.0, None, OP.add)

            ones_col = consts.tile([P, 1], F32)
            nc.vector.memset(ones_col[:], 1.0)
            ones_row1 = consts.tile([1, P], F32)
            nc.vector.memset(ones_row1[:], 1.0)

            # own-slice weights: a = exp(p), b = exp(-p), f32 + bf16
            ab = consts.tile([P, UT, 2], F32)
            nc.scalar.activation(ab[:, :, 0], pcol_sb, AF.Exp)
            nc.scalar.activation(ab[:, :, 1], pcol_sb, AF.Exp, scale=-1.0)
            ab_bf = consts.tile([P, UT, 2], BF16)
            nc.vector.tensor_copy(ab_bf[:], ab[:])

            for _rep in range(reps):
                # ---- phase A: quantize + local histogram ----
                # broadcast own t slice (row layout) early; used for OHlu
                t_b = work.tile([P, US], F32, tag="t_b")
                nc.sync.dma_start(
                    t_b[:], t_own.ap()[0:1, :].to_broadcast((P, US))
                )
                # column-side quantization (own 1024 elements, [128, 8])
                k_col = rp.tile([P, UT], I16, tag="k_col")
                nc.vector.tensor_scalar(
                    k_col[:], tcol_sb, SCALE, OFF, OP.mult, OP.add
                )
                kf_col = rp.tile([P, UT], F32, tag="kf_col")
                nc.vector.tensor_copy(kf_col[:], k_col[:])
                hi_col_i = rp.tile([P, UT], I16, tag="hi_col_i")
                nc.vector.tensor_scalar(
                    hi_col_i[:], kf_col[:], 1.0 / 128.0, -63.5 / 128.0,
                    OP.mult, OP.add,
                )
                hi_col = rp.tile([P, UT], F32, tag="hi_col")
                nc.vector.tensor_copy(hi_col[:], hi_col_i[:])
                lo_col_i = rp.tile([P, UT], I16, tag="lo_col_i")
                nc.vector.tensor_scalar(
                    lo_col_i[:], k_col[:], 127, None, OP.bitwise_and
                )
                lo_col = rp.tile([P, UT], F32, tag="lo_col")
                nc.vector.tensor_copy(lo_col[:], lo_col_i[:])

                # row-side quantization ([128, 1024] broadcast)
                k_row = rp.tile([P, US], I16, tag="k_row")
                nc.vector.tensor_scalar(
                    k_row[:], t_b[:], SCALE, OFF, OP.mult, OP.add
                )
                lo_row = rp.tile([P, US], I16, tag="lo_row")
                nc.vector.tensor_scalar(lo_row[:], k_row[:], 127, None, OP.bitwise_and)

                # j-side one-hots + G matmuls: G_T[l, h] accumulated in PSUM
                g_ps = psA.tile([P, P], F32, tag="g_ps")
                for c in range(UT):
                    ohh = work.tile([P, P], BF16, tag="ohh")
                    nc.vector.tensor_scalar(
                        ohh[:], iota_row_i[:], hi_col[:, c : c + 1], None,
                        OP.is_equal,
                    )
                    ohl = work.tile([P, P], BF16, tag="ohl")
                    nc.vector.tensor_scalar(
                        ohl[:], iota_row_i[:], lo_col[:, c : c + 1], None,
                        OP.is_equal,
                    )
                    nc.tensor.matmul(
                        g_ps[:], lhsT=ohl[:], rhs=ohh[:],
                        start=(c == 0), stop=(c == UT - 1),
                    )
                g_sb = rp.tile([P, P], F32, tag="g_sb")
                nc.vector.tensor_copy(g_sb[:], g_ps[:])

                # ---- collective 1: AllReduce G ----
                g_dram = dram.tile([P, P], F32, tag="g_dram")
                nc.sync.dma_start(g_dram[:], g_sb[:])
                ga_dram = dram.tile([P, P], F32, tag="ga_dram")
                if coll1:
                    nc.gpsimd.collective_compute(
                        "AllReduce",
                        OP.add,
                        replica_groups=[list(range(NCORE))],
                        ins=[g_dram[:].opt()],
                        outs=[ga_dram[:].opt()],
                    )
                else:  # timing-sim variant: stand-in DMA, wrong data
                    nc.sync.dma_start(ga_dram[:], g_dram[:])
                ga_sb = rp.tile([P, P], F32, tag="ga_sb")
                nc.sync.dma_start(ga_sb[:], ga_dram[:])
                if debug:
                    nc.sync.dma_start(dbg_g.ap(), ga_sb[:])

                # u-side one-hots (overlap with the collective)
                ohlu = rp.tile([P, UT, P], F32, tag="ohlu")
                ohhu = rp.tile([P, UT, P], BF16, tag="ohhu")
                for k in range(UT):
                    nc.vector.tensor_scalar(
                        ohlu[:, k, :], lo_row[:, k * P : (k + 1) * P],
                        iota_col_f[:], None, OP.is_equal,
                    )
                    nc.vector.tensor_scalar(
                        ohhu[:, k, :], iota_row_i[:], hi_col[:, k : k + 1],
                        None, OP.is_equal,
                    )

                # ---- rank tables (PE) ----
                sm_ps = psA.tile([P, 132], F32, tag="sm_ps")
                hc_ps = sm_ps[:, 0:1]
                nc.tensor.matmul(
                    hc_ps, lhsT=ga_sb[:], rhs=ones_col[:], start=True, stop=True
                )
                hc_sb = rp.tile([P, 1], F32, tag="hc_sb")
                nc.vector.tensor_copy(hc_sb[:], hc_ps)
                sufrow_ps = sm_ps[0:1, 4:132]
                nc.tensor.matmul(
                    sufrow_ps, lhsT=hc_sb[:], rhs=stri[:], start=True, stop=True
                )
                sufrow_sb = rp.tile([1, P], F32, tag="sufrow_sb")
                nc.vector.tensor_copy(sufrow_sb[:], sufrow_ps)
                rhs2_ps = psA.tile([P, P], F32, tag="rhs2_ps")
                nc.tensor.matmul(
                    rhs2_ps[:], lhsT=stri[:], rhs=ga_sb[:], start=True, stop=False
                )
                nc.tensor.matmul(
                    rhs2_ps[:], lhsT=ones_row1[:], rhs=sufrow_sb[:],
                    start=False, stop=True,
                )
                rhs2_sb = rp.tile([P, P], F32, tag="rhs2_sb")
                nc.vector.tensor_copy(rhs2_sb[:], rhs2_ps[:])

                # ---- per-element ranks: W2 matmul + hi-pick TTR ----
                rsum = rp.tile([P, UT], F32, tag="rsum")
                for k in range(UT):
                    w2_ps = psB.tile([P, P], F32, tag="w2_ps")
                    nc.tensor.matmul(
                        w2_ps[:], lhsT=ohlu[:, k, :], rhs=rhs2_sb[:],
                        start=True, stop=True,
                    )
                    scr = work.tile([P, P], F32, tag="scr_ttr")
                    nc.vector.tensor_tensor_reduce(
                        scr[:], w2_ps[:], ohhu[:, k, :], 1.0, 0.0,
                        OP.mult, OP.add, accum_out=rsum[:, k : k + 1],
                    )
                if debug:
                    nc.sync.dma_start(dbg_r.ap(), rsum[:])

                # ---- phase 1.5/2: m, step masks, F/SufH matmuls ----
                tmp = rp.tile([P, UT], F32, tag="tmp")
                m_col = rp.tile([P, UT], F32, tag="m_col")
                step = rp.tile([P, UT, NQ + 1], BF16, tag="step")
                qp1 = rp.tile([P, UT], F32, tag="qp1")
                sm_col = rp.tile([P, UT], F32, tag="sm_col")
                f_ps = psA.tile([P, 64], F32, tag="f_ps")
                h_ps = psA.tile([P, 2], F32, tag="h_ps")

                nc.vector.tensor_scalar(
                    tmp[:], rsum[:], float(N - 1), -1.0, OP.subtract, OP.mult
                )
                nc.vector.tensor_tensor(m_col[:], rsum[:], tmp[:], OP.min)
                if debug:
                    nc.sync.dma_start(dbg_m.ap(), m_col[:])
                for k in range(UT):
                    nc.vector.tensor_scalar(
                        step[:, k, :], iota32e[:], m_col[:, k : k + 1], None,
                        OP.is_le, OP.add, accum_out=qp1[:, k : k + 1],
                    )
                nc.vector.scalar_tensor_tensor(
                    sm_col[:], qp1[:], -32.0, m_col[:], OP.mult, OP.add
                )
                for k in range(UT):
                    oq = work.tile([P, NQ], BF16, tag="oq")
                    nc.vector.tensor_tensor(
                        oq[:], step[:, k, 0:NQ], step[:, k, 1 : NQ + 1],
                        OP.subtract,
                    )
                    ssab = work.tile([P, 2 * NS], BF16, tag="ssab")
                    nc.vector.tensor_scalar(
                        ssab[:, 0:NS], iotaSm[:], sm_col[:, k : k + 1],
                        ab[:, k, 0:1], OP.is_le, OP.mult,
                    )
                    nc.vector.tensor_scalar(
                        ssab[:, NS : 2 * NS], iotaSm[:], sm_col[:, k : k + 1],
                        ab[:, k, 1:2], OP.is_le, OP.mult,
                    )
                    nc.tensor.matmul(
                        f_ps[:], lhsT=oq[:], rhs=ssab[:],
                        start=(k == 0), stop=(k == UT - 1),
                    )
                    nc.tensor.matmul(
                        h_ps[:], lhsT=step[:, k, 1 : NQ + 1],
                        rhs=ab_bf[:, k, :],
                        start=(k == 0), stop=(k == UT - 1),
                    )

                # num partial: sum_u pred_u * (2*[r_u < N/2] - 1)
                sgn = rp.tile([P, UT], F32, tag="sgn")
                nc.vector.tensor_scalar(sgn[:], rsum[:], float(NPAIR), None, OP.is_lt)
                nc.vector.tensor_scalar(sgn[:], sgn[:], 2.0, -1.0, OP.mult, OP.add)
                xp = rp.tile([P, UT], F32, tag="xp")
                nc.vector.tensor_tensor(xp[:], sgn[:], pcol_sb, OP.mult)
                xq = rp.tile([P, 1], F32, tag="xq")
                nc.vector.tensor_reduce(
                    xq[:], xp[:], axis=mybir.AxisListType.X, op=OP.add
                )
                np_ps = sm_ps[0:1, 1:2]
                nc.tensor.matmul(
                    np_ps, lhsT=xq[:], rhs=ones_col[:], start=True, stop=True
                )

                fh_in = rp.tile([P, 67], F32, tag="fh_in")
                nc.vector.tensor_copy(fh_in[:, 0:64], f_ps[:])
                nc.vector.tensor_copy(fh_in[:, 64:66], h_ps[:])
                nc.vector.memset(fh_in[:, 66:67], 0.0)
                nc.vector.tensor_copy(fh_in[0:1, 66:67], np_ps)

                # ---- collective 2: ReduceScatter F/SufH/num ----
                QC = P // NCORE  # 16 Q rows per core
                fh_dram = dram.tile([P, 67], F32, tag="fh_dram")
                nc.sync.dma_start(fh_dram[:], fh_in[:])
                fhrs_dram = dram.tile([QC, 67], F32, tag="fhrs_dram")
                if coll2:
                    nc.gpsimd.collective_compute(
                        "ReduceScatter",
                        OP.add,
                        replica_groups=[list(range(NCORE))],
                        ins=[fh_dram[:].opt()],
                        outs=[fhrs_dram[:].opt()],
                    )
                else:
                    nc.sync.dma_start(fhrs_dram[:], fh_dram[0:QC, :])
                fh_sb = rp.tile([QC, 67], F32, tag="fh_sb")
                nc.sync.dma_start(fh_sb[:], fhrs_dram[:])
                if debug:
                    nc.sync.dma_start(dbg_fh.ap(), fh_sb[:])

                # ---- phase 3: denom/ln on this core's 512 windows ----
                at = rp.tile([QC, NS], F32, tag="at")
                nc.vector.tensor_scalar(
                    at[:], fh_sb[:, 0:NS], fh_sb[:, 64:65], None, OP.add
                )
                bt = rp.tile([QC, NS], F32, tag="bt")
                nc.vector.tensor_scalar(
                    bt[:], fh_sb[:, NS : 2 * NS], fh_sb[:, 65:66], None, OP.add
                )
                den = rp.tile([QC, NS], F32, tag="den")
                nc.vector.tensor_tensor(den[:], at[:], bt[:], OP.mult)
                nc.vector.tensor_tensor(den[:], den[:], win_sb[0:QC, :], OP.subtract)
                # quantization ties can empty the innermost window; mirror the
                # reference's where(denom <= 0, EPS, denom) guard
                nc.vector.tensor_scalar(den[:], den[:], 1e-8, None, OP.max)
                logd = rp.tile([QC, NS], F32, tag="logd")
                lnacc = rp.tile([QC, 1], F32, tag="lnacc")
                nc.scalar.activation(logd[:], den[:], AF.Ln, accum_out=lnacc[:])
                ln_ps = sm_ps[0:1, 2:3]
                nc.tensor.matmul(
                    ln_ps, lhsT=lnacc[:], rhs=ones_col[0:QC, :],
                    start=True, stop=True,
                )
                out_sb = rp.tile([1, 1], F32, tag="out_sb")
                nc.vector.tensor_tensor(
                    out_sb[:], ln_ps, fh_sb[0:1, 66:67], OP.subtract
                )
                nc.sync.dma_start(out_part.ap(), out_sb[:])

    nc.compile()
    return nc


def make_in_maps(pred: np.ndarray, target: np.ndarray):
    pred = np.ascontiguousarray(pred, dtype=np.float32).reshape(N)
    target = np.ascontiguousarray(target, dtype=np.float32).reshape(N)
    in_maps = []
    for c in range(NCORE):
        tsl = target[c * US : (c + 1) * US]
        psl = pred[c * US : (c + 1) * US]
        win = np.zeros((P, NS), np.float32)
        rho = np.arange(P // NCORE)
        s_i = np.arange(NS)
        win[: P // NCORE, :] = (
            N - 64.0 * (16 * c + rho)[:, None] - 2.0 * s_i[None, :]
        )
        pk = np.concatenate(
            [tsl.reshape(UT, P).T, psl.reshape(UT, P).T, win], axis=1
        ).astype(np.float32)
        in_maps.append(
            {
                "t_own": np.ascontiguousarray(tsl.reshape(1, US)),
                "packed": np.ascontiguousarray(pk),
            }
        )
    return in_maps


_CACHE = {}


def _get_module():
    if "nc" not in _CACHE:
        _CACHE["nc"] = build_module(debug=False)
    return _CACHE["nc"]


def kernel(pred: np.ndarray, target: np.ndarray) -> np.ndarray:
    from concourse import bass_utils

    nc = _get_module()
    in_maps = make_in_maps(pred, target)
    res = bass_utils.run_bass_kernel_spmd(nc, in_maps, core_ids=list(range(NCORE)))
    total = np.float32(0.0)
    for c in range(NCORE):
        total = np.float32(total + res.results[c]["out_part"][0, 0])
    return np.asarray(total, dtype=np.float32)


# revision 3
# speedup vs baseline: 3.4778x; 1.4530x over previous
"""ListFoldLoss Trainium2 kernel v1 (8-core SPMD, Bass/Tile).

Same math as the baseline (rank-1 factorization of the psi matrix; see
kernel.py docstring) but phase 1 (ranks) is computed via a quantized-key
histogram instead of N brute-force compares per element:

  key_u = round(t_u * SCALE + OFF) as int16 in [0, 16384)   (14 bits)
  hi = round((key - 63.5)/128)  (= floor(key/128)),  lo = key mod 128
  r_u = #{j: key_j > key_u} = SufCntHi[hi_u] + SufG_lo[hi_u, lo_u]

Per core: one-hot the own 1024 keys' digits (is_equal vs iota), PE-matmul
them into a local 2D count histogram G_T[l, h]; AllReduce G (64KB); build
  HiCnt[h]    = sum_l G_T[l, h]                    (PE: G_T^T @ ones)
  SufRow[h]   = sum_{h'>h} HiCnt[h']               (PE: HC^T @ STRI)
  rhs2[l', h] = sum_{l>l'} G_T[l, h] + SufRow[h]   (PE: STRI^T@G_T + outer)
then per u-subtile W2 = OHlu^T @ rhs2 gives row u = SufG_lo[:, lo_u] +
SufRow[:], and a tensor_tensor_reduce pick against the hi one-hot yields
r_u exactly.  Quantization ties (elements sharing a 5.4e-4-wide key bin get
equal ranks) perturb the loss like the baseline's bf16 ties (~1e-4 rel).

Phases 1.5/2/3 are the baseline's: m = min(r, N-1-r), 32-grid step masks,
F/SufH PSUM matmuls, ReduceScatter [128,67] -> [16,67], per-core denom/ln,
host-summed partials.
"""

import numpy as np

import concourse.bacc as bacc
import concourse.bass as bass
import concourse.mybir as mybir
import concourse.tile as tile

N = 8192
NCORE = 8
P = 128
US = N // NCORE          # 1024 u's per core
UT = US // P             # 8 u-subtiles per core
NPAIR = N // 2           # 4096 loss terms
NQ = 128                 # coarse window blocks (i = 32Q + S)
NS = 32

SCALE = 1836.0
OFF = 8192.0

F32 = mybir.dt.float32
BF16 = mybir.dt.bfloat16
F16 = mybir.dt.float16
I16 = mybir.dt.int16
AF = mybir.ActivationFunctionType
OP = mybir.AluOpType


def build_module(
    debug: bool = False,
    reps: int = 1,
    collective: bool = True,
    work_bufs: int = 2,
    coll1: bool | None = None,
    coll2: bool | None = None,
):
    coll1 = collective if coll1 is None else coll1
    coll2 = collective if coll2 is None else coll2
    nc = bacc.Bacc(
        "TRN2",
        target_bir_lowering=False,
        debug=False,
        enable_asserts=False,
        num_devices=NCORE,
    )

    t_own = nc.dram_tensor("t_own", [1, US], F32, kind="ExternalInput")
    tqcol = nc.dram_tensor("tqcol", [P, UT], F32, kind="ExternalInput")
    # packed small consts: [tcol 8 | pcol 8 | win 32] per partition
    NPK = 2 * UT + NS
    packed = nc.dram_tensor("packed", [P, NPK], F32, kind="ExternalInput")
    out_part = nc.dram_tensor("out_part", [1, 1], F32, kind="ExternalOutput")
    if debug:
        dbg_r = nc.dram_tensor("dbg_r", [P, UT], F32, kind="ExternalOutput")
        dbg_m = nc.dram_tensor("dbg_m", [P, UT], F32, kind="ExternalOutput")
        dbg_g = nc.dram_tensor("dbg_g", [P, P], F16, kind="ExternalOutput")
        dbg_fh = nc.dram_tensor("dbg_fh", [P // NCORE, 67], F32, kind="ExternalOutput")

    with tile.TileContext(nc) as tc:
        with (
            tc.tile_pool(name="consts", bufs=1) as consts,
            tc.tile_pool(name="rep", bufs=2) as rp,
            tc.tile_pool(name="work", bufs=work_bufs) as work,
            tc.tile_pool(name="psA", bufs=1, space="PSUM") as psA,
            tc.tile_pool(name="psB", bufs=2, space="PSUM") as psB,
            tc.tile_pool(name="dram", bufs=2, space="DRAM") as dram,
        ):
            # ---- constant/small loads ----
            tq_sb = consts.tile([P, UT], F32)
            nc.sync.dma_start(tq_sb[:], tqcol.ap())
            packed_sb = consts.tile([P, NPK], F32)
            nc.sync.dma_start(packed_sb[:], packed.ap())
            tcol_sb = tq_sb[:]
            pcol_sb = packed_sb[:, UT : 2 * UT]
            win_sb = packed_sb[:, 2 * UT : 2 * UT + NS]

            # on-device iotas / masks (init-time only)
            iota_row_i = consts.tile([P, P], I16)
            nc.gpsimd.iota(iota_row_i[:], [[1, P]], base=0, channel_multiplier=0)
            iota_col_i = consts.tile([P, 1], I16)
            nc.gpsimd.iota(iota_col_i[:], [[0, 1]], base=0, channel_multiplier=1)
            iota_row_f = consts.tile([P, P], F32)
            nc.vector.tensor_copy(iota_row_f[:], iota_row_i[:])
            iota_col_f = consts.tile([P, 1], F32)
            nc.vector.tensor_copy(iota_col_f[:], iota_col_i[:])
            # STRI[p, f] = [p > f]  (strict lower in (p, f)); f32 for fp32 matmuls
            stri = consts.tile([P, P], F32)
            nc.vector.tensor_scalar(
                stri[:], iota_row_f[:], iota_col_f[:], None, OP.is_lt
            )

            is_i = consts.tile([P, NS], I16)
            nc.gpsimd.iota(is_i[:], [[1, NS]], base=0, channel_multiplier=0)
            iotaS = consts.tile([P, NS], F32)
            nc.vector.tensor_copy(iotaS[:], is_i[:])
            i32_i = consts.tile([P, NQ + 1], I16)
            nc.gpsimd.iota(i32_i[:], [[32, NQ + 1]], base=0, channel_multiplier=0)
            iota32e = consts.tile([P, NQ + 1], F32)
            nc.vector.tensor_copy(iota32e[:], i32_i[:])
            iotaSm = consts.tile([P, NS], F32)
            nc.vector.tensor_scalar(iotaSm[:], iotaS[:], -32.0, None, OP.add)

            stri_bf = consts.tile([P, P], BF16)
            nc.vector.tensor_copy(stri_bf[:], stri[:])
            ones_col = consts.tile([P, 1], F32)
            nc.vector.memset(ones_col[:], 1.0)
            ones_col_bf = consts.tile([P, 1], BF16)
            nc.vector.memset(ones_col_bf[:], 1.0)
            ones_row1 = consts.tile([1, P], F32)
            nc.vector.memset(ones_row1[:], 1.0)

            # own-slice weights: a = exp(p), b = exp(-p), f32 + bf16
            ab = consts.tile([P, UT, 2], F32)
            nc.scalar.activation(ab[:, :, 0], pcol_sb, AF.Exp)
            nc.scalar.activation(ab[:, :, 1], pcol_sb, AF.Exp, scale=-1.0)
            ab_bf = consts.tile([P, UT, 2], BF16)
            nc.vector.tensor_copy(ab_bf[:], ab[:])

            for _rep in range(reps):
                # ---- phase A: quantize + local histogram ----
                # broadcast own t slice (row layout) early; used for OHlu
                t_b = work.tile([P, US], F32, tag="t_b")
                nc.sync.dma_start(
                    t_b[:], t_own.ap()[0:1, :].to_broadcast((P, US))
                )
                # column-side quantization (own 1024 elements, [128, 8])
                k_col = rp.tile([P, UT], I16, tag="k_col")
                nc.vector.tensor_scalar(
                    k_col[:], tcol_sb, SCALE, OFF, OP.mult, OP.add
                )
                kf_col = rp.tile([P, UT], F32, tag="kf_col")
                nc.vector.tensor_copy(kf_col[:], k_col[:])
                hi_col_i = rp.tile([P, UT], I16, tag="hi_col_i")
                nc.vector.tensor_scalar(
                    hi_col_i[:], kf_col[:], 1.0 / 128.0, -63.5 / 128.0,
                    OP.mult, OP.add,
                )
                hi_col = rp.tile([P, UT], F32, tag="hi_col")
                nc.vector.tensor_copy(hi_col[:], hi_col_i[:])
                lo_col_i = rp.tile([P, UT], I16, tag="lo_col_i")
                nc.vector.tensor_scalar(
                    lo_col_i[:], k_col[:], 127, None, OP.bitwise_and
                )
                lo_col = rp.tile([P, UT], F32, tag="lo_col")
                nc.vector.tensor_copy(lo_col[:], lo_col_i[:])

                # j-side one-hots + G matmuls: G_T[l, h] accumulated in PSUM
                g_ps = psA.tile([P, P], F32, tag="g_ps")
                for c in range(UT):
                    ohh = work.tile([P, P], BF16, tag="ohh")
                    nc.vector.tensor_scalar(
                        ohh[:], iota_row_i[:], hi_col[:, c : c + 1], None,
                        OP.is_equal,
                    )
                    ohl = work.tile([P, P], BF16, tag="ohl")
                    nc.vector.tensor_scalar(
                        ohl[:], iota_row_i[:], lo_col[:, c : c + 1], None,
                        OP.is_equal,
                    )
                    nc.tensor.matmul(
                        g_ps[:], lhsT=ohl[:], rhs=ohh[:],
                        start=(c == 0), stop=(c == UT - 1),
                    )
                g_sb = rp.tile([P, P], F16, tag="g_sb")
                nc.vector.tensor_copy(g_sb[:], g_ps[:])

                # ---- collective 1: AllReduce G (f16: counts <= 2048 exact) ----
                g_dram = dram.tile([P, P], F16, tag="g_dram")
                nc.sync.dma_start(g_dram[:], g_sb[:])

                # row-side quantization ([128, 1024] broadcast)
                k_row = rp.tile([P, US], I16, tag="k_row")
                nc.vector.tensor_scalar(
                    k_row[:], t_b[:], SCALE, OFF, OP.mult, OP.add
                )
                lo_row = rp.tile([P, US], I16, tag="lo_row")
                nc.vector.tensor_scalar(lo_row[:], k_row[:], 127, None, OP.bitwise_and)
                ga_dram = dram.tile([P, P], F16, tag="ga_dram")
                if coll1:
                    nc.gpsimd.collective_compute(
                        "AllReduce",
                        OP.add,
                        replica_groups=[list(range(NCORE))],
                        ins=[g_dram[:].opt()],
                        outs=[ga_dram[:].opt()],
                    )
                else:  # timing-sim variant: stand-in DMA, wrong data
                    nc.sync.dma_start(ga_dram[:], g_dram[:])
                ga_sb = rp.tile([P, P], F16, tag="ga_sb")
                nc.sync.dma_start(ga_sb[:], ga_dram[:])
                if debug:
                    nc.sync.dma_start(dbg_g.ap(), ga_sb[:])

                # u-side one-hots (overlap with the collective)
                ohlu = rp.tile([P, UT, P], F32, tag="ohlu")
                ohhu = rp.tile([P, UT, P], BF16, tag="ohhu")
                for k in range(UT):
                    nc.vector.tensor_scalar(
                        ohlu[:, k, :], lo_row[:, k * P : (k + 1) * P],
                        iota_col_f[:], None, OP.is_equal,
                    )
                    nc.vector.tensor_scalar(
                        ohhu[:, k, :], iota_row_i[:], hi_col[:, k : k + 1],
                        None, OP.is_equal,
                    )

                # ---- rank tables (PE) ----
                sm_ps = psA.tile([P, 132], F32, tag="sm_ps")
                hc_ps = sm_ps[:, 0:1]
                nc.tensor.matmul(
                    hc_ps, lhsT=ga_sb[:], rhs=ones_col[:], start=True, stop=True
                )
                hc_sb = rp.tile([P, 1], F32, tag="hc_sb")
                nc.vector.tensor_copy(hc_sb[:], hc_ps)
                sufrow_ps = sm_ps[0:1, 4:132]
                nc.tensor.matmul(
                    sufrow_ps, lhsT=hc_sb[:], rhs=stri[:], start=True, stop=True
                )
                sufrow_sb = rp.tile([1, P], F32, tag="sufrow_sb")
                nc.vector.tensor_copy(sufrow_sb[:], sufrow_ps)
                rhs2_ps = psA.tile([P, P], F32, tag="rhs2_ps")
                nc.tensor.matmul(
                    rhs2_ps[:], lhsT=stri[:], rhs=ga_sb[:], start=True, stop=False
                )
                nc.tensor.matmul(
                    rhs2_ps[:], lhsT=ones_row1[:], rhs=sufrow_sb[:],
                    start=False, stop=True,
                )
                rhs2_sb = rp.tile([P, P], F32, tag="rhs2_sb")
                nc.vector.tensor_copy(rhs2_sb[:], rhs2_ps[:])

                # ---- per-element ranks: W2 matmul + hi-pick TTR ----
                rsum = rp.tile([P, UT], F32, tag="rsum")
                for k in range(UT):
                    w2_ps = psB.tile([P, P], F32, tag="w2_ps")
                    nc.tensor.matmul(
                        w2_ps[:], lhsT=ohlu[:, k, :], rhs=rhs2_sb[:],
                        start=True, stop=True,
                    )
                    scr = work.tile([P, P], F32, tag="scr_ttr")
                    nc.vector.tensor_tensor_reduce(
                        scr[:], w2_ps[:], ohhu[:, k, :], 1.0, 0.0,
                        OP.mult, OP.add, accum_out=rsum[:, k : k + 1],
                    )
                if debug:
                    nc.sync.dma_start(dbg_r.ap(), rsum[:])

                # ---- phase 1.5/2: m, q one-hots, F + Q-hist matmuls ----
                # q = floor(m/32) via round((m-15.5)/32); s = m - 32q
                # OQ[u, Q] = [q_u == Q]; F += OQ^T @ ssab;
                # Hq[Q, 2] += OQ^T @ ab;  SufH = STRI^T @ Hq (strict suffix)
                tmp = rp.tile([P, UT], F32, tag="tmp")
                m_col = rp.tile([P, UT], F32, tag="m_col")
                q_col_i = rp.tile([P, UT], I16, tag="q_col_i")
                q_col = rp.tile([P, UT], F32, tag="q_col")
                s_col = rp.tile([P, UT], F32, tag="s_col")
                f_ps = psA.tile([P, 64], F32, tag="f_ps")
                h_ps = psA.tile([P, 2], F32, tag="h_ps")

                nc.vector.tensor_scalar(
                    tmp[:], rsum[:], float(N - 1), -1.0, OP.subtract, OP.mult
                )
                nc.vector.tensor_tensor(m_col[:], rsum[:], tmp[:], OP.min)
                if debug:
                    nc.sync.dma_start(dbg_m.ap(), m_col[:])
                nc.vector.tensor_scalar(
                    q_col_i[:], m_col[:], 1.0 / 32.0, -15.5 / 32.0,
                    OP.mult, OP.add,
                )
                nc.vector.tensor_copy(q_col[:], q_col_i[:])
                nc.vector.scalar_tensor_tensor(
                    s_col[:], q_col[:], -32.0, m_col[:], OP.mult, OP.add
                )
                hq_ps = psA.tile([P, 2], F32, tag="hq_ps")
                for k in range(UT):
                    oq = work.tile([P, NQ], BF16, tag="oq")
                    nc.vector.tensor_scalar(
                        oq[:], iota_row_f[:], q_col[:, k : k + 1], None,
                        OP.is_equal,
                    )
                    ssab = work.tile([P, 2 * NS], BF16, tag="ssab")
                    nc.vector.tensor_scalar(
                        ssab[:, 0:NS], iotaS[:], s_col[:, k : k + 1],
                        ab[:, k, 0:1], OP.is_le, OP.mult,
                    )
                    nc.vector.tensor_scalar(
                        ssab[:, NS : 2 * NS], iotaS[:], s_col[:, k : k + 1],
                        ab[:, k, 1:2], OP.is_le, OP.mult,
                    )
                    nc.tensor.matmul(
                        f_ps[:], lhsT=oq[:], rhs=ssab[:],
                        start=(k == 0), stop=(k == UT - 1),
                    )
                    nc.tensor.matmul(
                        hq_ps[:], lhsT=oq[:], rhs=ab_bf[:, k, :],
                        start=(k == 0), stop=(k == UT - 1),
                    )
                hq_sb = rp.tile([P, 2], F32, tag="hq_sb")
                nc.vector.tensor_copy(hq_sb[:], hq_ps[:])
                nc.tensor.matmul(
                    h_ps[:], lhsT=stri[:], rhs=hq_sb[:], start=True, stop=True
                )

                # num partial: sum_u pred_u * (2*[r_u < N/2] - 1)
                sgn = rp.tile([P, UT], F32, tag="sgn")
                nc.vector.tensor_scalar(sgn[:], rsum[:], float(NPAIR), None, OP.is_lt)
                nc.vector.tensor_scalar(sgn[:], sgn[:], 2.0, -1.0, OP.mult, OP.add)
                xp = rp.tile([P, UT], F32, tag="xp")
                nc.vector.tensor_tensor(xp[:], sgn[:], pcol_sb, OP.mult)
                xq = rp.tile([P, 1], F32, tag="xq")
                nc.vector.tensor_reduce(
                    xq[:], xp[:], axis=mybir.AxisListType.X, op=OP.add
                )
                np_ps = sm_ps[0:1, 1:2]
                nc.tensor.matmul(
                    np_ps, lhsT=xq[:], rhs=ones_col[:], start=True, stop=True
                )

                fh_in = rp.tile([P, 67], F32, tag="fh_in")
                nc.vector.tensor_copy(fh_in[:, 0:64], f_ps[:])
                nc.vector.tensor_copy(fh_in[:, 64:66], h_ps[:])
                nc.vector.memset(fh_in[:, 66:67], 0.0)
                nc.vector.tensor_copy(fh_in[0:1, 66:67], np_ps)

                # ---- collective 2: ReduceScatter F/SufH/num ----
                QC = P // NCORE  # 16 Q rows per core
                fh_dram = dram.tile([P, 67], F32, tag="fh_dram")
                nc.sync.dma_start(fh_dram[:], fh_in[:])
                fhrs_dram = dram.tile([QC, 67], F32, tag="fhrs_dram")
                if coll2:
                    nc.gpsimd.collective_compute(
                        "ReduceScatter",
                        OP.add,
                        replica_groups=[list(range(NCORE))],
                        ins=[fh_dram[:].opt()],
                        outs=[fhrs_dram[:].opt()],
                    )
                else:
                    nc.sync.dma_start(fhrs_dram[:], fh_dram[0:QC, :])
                fh_sb = rp.tile([QC, 67], F32, tag="fh_sb")
                nc.sync.dma_start(fh_sb[:], fhrs_dram[:])
                if debug:
                    nc.sync.dma_start(dbg_fh.ap(), fh_sb[:])

                # ---- phase 3: denom/ln on this core's 512 windows ----
                at = rp.tile([QC, NS], F32, tag="at")
                nc.vector.tensor_scalar(
                    at[:], fh_sb[:, 0:NS], fh_sb[:, 64:65], None, OP.add
                )
                bt = rp.tile([QC, NS], F32, tag="bt")
                nc.vector.tensor_scalar(
                    bt[:], fh_sb[:, NS : 2 * NS], fh_sb[:, 65:66], None, OP.add
                )
                den = rp.tile([QC, NS], F32, tag="den")
                nc.vector.tensor_tensor(den[:], at[:], bt[:], OP.mult)
                nc.vector.tensor_tensor(den[:], den[:], win_sb[0:QC, :], OP.subtract)
                # quantization ties can empty the innermost window; mirror the
                # reference's where(denom <= 0, EPS, denom) guard
                nc.vector.tensor_scalar(den[:], den[:], 1e-8, None, OP.max)
                logd = rp.tile([QC, NS], F32, tag="logd")
                lnacc = rp.tile([QC, 1], F32, tag="lnacc")
                nc.scalar.activation(logd[:], den[:], AF.Ln, accum_out=lnacc[:])
                ln_ps = sm_ps[0:1, 2:3]
                nc.tensor.matmul(
                    ln_ps, lhsT=lnacc[:], rhs=ones_col[0:QC, :],
                    start=True, stop=True,
                )
                out_sb = rp.tile([1, 1], F32, tag="out_sb")
                nc.vector.tensor_tensor(
                    out_sb[:], ln_ps, fh_sb[0:1, 66:67], OP.subtract
                )
                nc.sync.dma_start(out_part.ap(), out_sb[:])

    nc.compile()
    return nc


def make_in_maps(pred: np.ndarray, target: np.ndarray):
    pred = np.ascontiguousarray(pred, dtype=np.float32).reshape(N)
    target = np.ascontiguousarray(target, dtype=np.float32).reshape(N)
    in_maps = []
    for c in range(NCORE):
        tsl = target[c * US : (c + 1) * US]
        psl = pred[c * US : (c + 1) * US]
        win = np.zeros((P, NS), np.float32)
        rho = np.arange(P // NCORE)
        s_i = np.arange(NS)
        win[: P // NCORE, :] = (
            N - 64.0 * (16 * c + rho)[:, None] - 2.0 * s_i[None, :]
        )
        pk = np.concatenate(
            [tsl.reshape(UT, P).T, psl.reshape(UT, P).T, win], axis=1
        ).astype(np.float32)
        in_maps.append(
            {
                "t_own": np.ascontiguousarray(tsl.reshape(1, US)),
                "tqcol": np.ascontiguousarray(tsl.reshape(UT, P).T),
                "packed": np.ascontiguousarray(pk),
            }
        )
    return in_maps


_CACHE = {}


def _get_module():
    if "nc" not in _CACHE:
        _CACHE["nc"] = build_module(debug=False)
    return _CACHE["nc"]


def kernel(pred: np.ndarray, target: np.ndarray) -> np.ndarray:
    from concourse import bass_utils

    nc = _get_module()
    in_maps = make_in_maps(pred, target)
    res = bass_utils.run_bass_kernel_spmd(nc, in_maps, core_ids=list(range(NCORE)))
    total = np.float32(0.0)
    for c in range(NCORE):
        total = np.float32(total + res.results[c]["out_part"][0, 0])
    return np.asarray(total, dtype=np.float32)


# revision 4
# speedup vs baseline: 4.3809x; 1.2597x over previous
"""ListFoldLoss Trainium2 kernel v1 (8-core SPMD, Bass/Tile).

Same math as the baseline (rank-1 factorization of the psi matrix; see
kernel.py docstring) but phase 1 (ranks) is computed via a quantized-key
histogram instead of N brute-force compares per element:

  key_u = round(t_u * SCALE + OFF) as int16 in [0, 16384)   (14 bits)
  hi = round((key - 63.5)/128)  (= floor(key/128)),  lo = key mod 128
  r_u = #{j: key_j > key_u} = SufCntHi[hi_u] + SufG_lo[hi_u, lo_u]

Per core: one-hot the own 1024 keys' digits (is_equal vs iota), PE-matmul
them into a local 2D count histogram G_T[l, h]; AllReduce G (64KB); build
  HiCnt[h]    = sum_l G_T[l, h]                    (PE: G_T^T @ ones)
  SufRow[h]   = sum_{h'>h} HiCnt[h']               (PE: HC^T @ STRI)
  rhs2[l', h] = sum_{l>l'} G_T[l, h] + SufRow[h]   (PE: STRI^T@G_T + outer)
then per u-subtile W2 = OHlu^T @ rhs2 gives row u = SufG_lo[:, lo_u] +
SufRow[:], and a tensor_tensor_reduce pick against the hi one-hot yields
r_u exactly.  Quantization ties (elements sharing a 5.4e-4-wide key bin get
equal ranks) perturb the loss like the baseline's bf16 ties (~1e-4 rel).

Phases 1.5/2/3 are the baseline's: m = min(r, N-1-r), 32-grid step masks,
F/SufH PSUM matmuls, ReduceScatter [128,67] -> [16,67], per-core denom/ln,
host-summed partials.
"""

import numpy as np

import concourse.bacc as bacc
import concourse.bass as bass
import concourse.mybir as mybir
import concourse.tile as tile

N = 8192
NCORE = 8
P = 128
US = N // NCORE          # 1024 u's per core
UT = US // P             # 8 u-subtiles per core
NPAIR = N // 2           # 4096 loss terms
NQ = 128                 # coarse window blocks (i = 32Q + S)
NS = 32

SCALE = 1836.0
OFF = 8192.0

F32 = mybir.dt.float32
BF16 = mybir.dt.bfloat16
F16 = mybir.dt.float16
I16 = mybir.dt.int16
AF = mybir.ActivationFunctionType
OP = mybir.AluOpType


def build_module(
    debug: bool = False,
    reps: int = 1,
    collective: bool = True,
    work_bufs: int = 2,
    coll1: bool | None = None,
    coll2: bool | None = None,
):
    coll1 = collective if coll1 is None else coll1
    coll2 = collective if coll2 is None else coll2
    nc = bacc.Bacc(
        "TRN2",
        target_bir_lowering=False,
        debug=False,
        enable_asserts=False,
        num_devices=NCORE,
    )

    t_own = nc.dram_tensor("t_own", [1, US], F32, kind="ExternalInput")
    tqcol = nc.dram_tensor("tqcol", [P, UT], F32, kind="ExternalInput")
    # packed small consts: [tcol 8 | pcol 8 | win 32] per partition
    NPK = 2 * UT + NS
    packed = nc.dram_tensor("packed", [P, NPK], F32, kind="ExternalInput")
    out_part = nc.dram_tensor("out_part", [1, 1], F32, kind="ExternalOutput")
    if debug:
        dbg_r = nc.dram_tensor("dbg_r", [P, UT], F32, kind="ExternalOutput")
        dbg_m = nc.dram_tensor("dbg_m", [P, UT], F32, kind="ExternalOutput")
        dbg_g = nc.dram_tensor("dbg_g", [P, P], F16, kind="ExternalOutput")
        dbg_fh = nc.dram_tensor("dbg_fh", [P // NCORE, 67], F16, kind="ExternalOutput")

    with tile.TileContext(nc) as tc:
        with (
            tc.tile_pool(name="consts", bufs=1) as consts,
            tc.tile_pool(name="rep", bufs=2) as rp,
            tc.tile_pool(name="work", bufs=work_bufs) as work,
            tc.tile_pool(name="psA", bufs=1, space="PSUM") as psA,
            tc.tile_pool(name="psB", bufs=2, space="PSUM") as psB,
            tc.tile_pool(name="dram", bufs=2, space="DRAM") as dram,
        ):
            # ---- constant/small loads ----
            tq_sb = consts.tile([P, UT], F32)
            nc.sync.dma_start(tq_sb[:], tqcol.ap())
            packed_sb = consts.tile([P, NPK], F32)
            nc.sync.dma_start(packed_sb[:], packed.ap())
            tcol_sb = tq_sb[:]
            pcol_sb = packed_sb[:, UT : 2 * UT]
            win_sb = packed_sb[:, 2 * UT : 2 * UT + NS]

            # on-device iotas / masks (init-time only)
            iota_row_i = consts.tile([P, P], I16)
            nc.gpsimd.iota(iota_row_i[:], [[1, P]], base=0, channel_multiplier=0)
            iota_col_i = consts.tile([P, 1], I16)
            nc.gpsimd.iota(iota_col_i[:], [[0, 1]], base=0, channel_multiplier=1)
            iota_row_f = consts.tile([P, P], F32)
            nc.vector.tensor_copy(iota_row_f[:], iota_row_i[:])
            iota_col_f = consts.tile([P, 1], F32)
            nc.vector.tensor_copy(iota_col_f[:], iota_col_i[:])
            # STRI[p, f] = [p > f]  (strict lower in (p, f)); f32 for fp32 matmuls
            stri = consts.tile([P, P], F32)
            nc.vector.tensor_scalar(
                stri[:], iota_row_f[:], iota_col_f[:], None, OP.is_lt
            )

            is_i = consts.tile([P, NS], I16)
            nc.gpsimd.iota(is_i[:], [[1, NS]], base=0, channel_multiplier=0)
            iotaS = consts.tile([P, NS], F32)
            nc.vector.tensor_copy(iotaS[:], is_i[:])
            i32_i = consts.tile([P, NQ + 1], I16)
            nc.gpsimd.iota(i32_i[:], [[32, NQ + 1]], base=0, channel_multiplier=0)
            iota32e = consts.tile([P, NQ + 1], F32)
            nc.vector.tensor_copy(iota32e[:], i32_i[:])
            iotaSm = consts.tile([P, NS], F32)
            nc.vector.tensor_scalar(iotaSm[:], iotaS[:], -32.0, None, OP.add)

            stri_bf = consts.tile([P, P], BF16)
            nc.vector.tensor_copy(stri_bf[:], stri[:])
            ones_col = consts.tile([P, 1], F32)
            nc.vector.memset(ones_col[:], 1.0)
            ones_col_bf = consts.tile([P, 1], BF16)
            nc.vector.memset(ones_col_bf[:], 1.0)
            ones_row1 = consts.tile([1, P], F32)
            nc.vector.memset(ones_row1[:], 1.0)

            # own-slice weights: a = exp(p), b = exp(-p), f32 + bf16
            ab = consts.tile([P, UT, 2], F32)
            nc.scalar.activation(ab[:, :, 0], pcol_sb, AF.Exp)
            nc.scalar.activation(ab[:, :, 1], pcol_sb, AF.Exp, scale=-1.0)
            ab_bf = consts.tile([P, UT, 2], BF16)
            nc.vector.tensor_copy(ab_bf[:], ab[:])

            for _rep in range(reps):
                # ---- phase A: quantize + local histogram ----
                # broadcast own t slice (row layout) early; used for OHlu
                t_b = work.tile([P, US], F32, tag="t_b")
                nc.sync.dma_start(
                    t_b[:], t_own.ap()[0:1, :].to_broadcast((P, US))
                )
                # column-side quantization (own 1024 elements, [128, 8])
                k_col = rp.tile([P, UT], I16, tag="k_col")
                nc.vector.tensor_scalar(
                    k_col[:], tcol_sb, SCALE, OFF, OP.mult, OP.add
                )
                kf_col = rp.tile([P, UT], F32, tag="kf_col")
                nc.vector.tensor_copy(kf_col[:], k_col[:])
                hi_col_i = rp.tile([P, UT], I16, tag="hi_col_i")
                nc.vector.tensor_scalar(
                    hi_col_i[:], kf_col[:], 1.0 / 128.0, -63.5 / 128.0,
                    OP.mult, OP.add,
                )
                hi_col = rp.tile([P, UT], F32, tag="hi_col")
                nc.vector.tensor_copy(hi_col[:], hi_col_i[:])
                lo_col_i = rp.tile([P, UT], I16, tag="lo_col_i")
                nc.vector.tensor_scalar(
                    lo_col_i[:], k_col[:], 127, None, OP.bitwise_and
                )
                lo_col = rp.tile([P, UT], F32, tag="lo_col")
                nc.vector.tensor_copy(lo_col[:], lo_col_i[:])

                # j-side one-hots + G matmuls: G_T[l, h] accumulated in PSUM
                g_ps = psA.tile([P, P], F32, tag="g_ps")
                for c in range(UT):
                    ohh = work.tile([P, P], BF16, tag="ohh")
                    nc.vector.tensor_scalar(
                        ohh[:], iota_row_i[:], hi_col[:, c : c + 1], None,
                        OP.is_equal,
                    )
                    ohl = work.tile([P, P], BF16, tag="ohl")
                    nc.vector.tensor_scalar(
                        ohl[:], iota_row_i[:], lo_col[:, c : c + 1], None,
                        OP.is_equal,
                    )
                    nc.tensor.matmul(
                        g_ps[:], lhsT=ohl[:], rhs=ohh[:],
                        start=(c == 0), stop=(c == UT - 1),
                    )
                g_sb = rp.tile([P, P], F16, tag="g_sb")
                nc.vector.tensor_copy(g_sb[:], g_ps[:])

                # ---- collective 1: AllReduce G (f16: counts <= 2048 exact) ----
                g_dram = dram.tile([P, P], F16, tag="g_dram")
                nc.sync.dma_start(g_dram[:], g_sb[:])

                # row-side quantization ([128, 1024] broadcast)
                k_row = rp.tile([P, US], I16, tag="k_row")
                nc.vector.tensor_scalar(
                    k_row[:], t_b[:], SCALE, OFF, OP.mult, OP.add
                )
                lo_row = rp.tile([P, US], I16, tag="lo_row")
                nc.vector.tensor_scalar(lo_row[:], k_row[:], 127, None, OP.bitwise_and)
                ga_dram = dram.tile([P, P], F16, tag="ga_dram")
                if coll1:
                    nc.gpsimd.collective_compute(
                        "AllReduce",
                        OP.add,
                        replica_groups=[list(range(NCORE))],
                        ins=[g_dram[:].opt()],
                        outs=[ga_dram[:].opt()],
                    )
                else:  # timing-sim variant: stand-in DMA, wrong data
                    nc.sync.dma_start(ga_dram[:], g_dram[:])
                ga_sb = rp.tile([P, P], F16, tag="ga_sb")
                nc.sync.dma_start(ga_sb[:], ga_dram[:])
                if debug:
                    nc.sync.dma_start(dbg_g.ap(), ga_sb[:])

                # u-side one-hots (overlap with the collective)
                ohlu = rp.tile([P, UT, P], F32, tag="ohlu")
                ohhu = rp.tile([P, UT, P], BF16, tag="ohhu")
                for k in range(UT):
                    nc.vector.tensor_scalar(
                        ohlu[:, k, :], lo_row[:, k * P : (k + 1) * P],
                        iota_col_f[:], None, OP.is_equal,
                    )
                    nc.vector.tensor_scalar(
                        ohhu[:, k, :], iota_row_i[:], hi_col[:, k : k + 1],
                        None, OP.is_equal,
                    )

                # ---- rank tables (PE) ----
                sm_ps = psA.tile([P, 132], F32, tag="sm_ps")
                hc_ps = sm_ps[:, 0:1]
                nc.tensor.matmul(
                    hc_ps, lhsT=ga_sb[:], rhs=ones_col[:], start=True, stop=True
                )
                hc_sb = rp.tile([P, 1], F32, tag="hc_sb")
                nc.vector.tensor_copy(hc_sb[:], hc_ps)
                sufrow_ps = sm_ps[0:1, 4:132]
                nc.tensor.matmul(
                    sufrow_ps, lhsT=hc_sb[:], rhs=stri[:], start=True, stop=True
                )
                sufrow_sb = rp.tile([1, P], F32, tag="sufrow_sb")
                nc.vector.tensor_copy(sufrow_sb[:], sufrow_ps)
                rhs2_ps = psA.tile([P, P], F32, tag="rhs2_ps")
                nc.tensor.matmul(
                    rhs2_ps[:], lhsT=stri[:], rhs=ga_sb[:], start=True, stop=False
                )
                nc.tensor.matmul(
                    rhs2_ps[:], lhsT=ones_row1[:], rhs=sufrow_sb[:],
                    start=False, stop=True,
                )
                rhs2_sb = rp.tile([P, P], F32, tag="rhs2_sb")
                nc.vector.tensor_copy(rhs2_sb[:], rhs2_ps[:])

                # ---- per-element ranks: W2 matmul + hi-pick TTR ----
                rsum = rp.tile([P, UT], F32, tag="rsum")
                for k in range(UT):
                    w2_ps = psB.tile([P, P], F32, tag="w2_ps")
                    nc.tensor.matmul(
                        w2_ps[:], lhsT=ohlu[:, k, :], rhs=rhs2_sb[:],
                        start=True, stop=True,
                    )
                    scr = work.tile([P, P], F32, tag="scr_ttr")
                    nc.vector.tensor_tensor_reduce(
                        scr[:], w2_ps[:], ohhu[:, k, :], 1.0, 0.0,
                        OP.mult, OP.add, accum_out=rsum[:, k : k + 1],
                    )
                if debug:
                    nc.sync.dma_start(dbg_r.ap(), rsum[:])

                # ---- phase 1.5/2: m, q one-hots, F + Q-hist matmuls ----
                # q = floor(m/32) via round((m-15.5)/32); s = m - 32q
                # OQ[u, Q] = [q_u == Q]; F += OQ^T @ ssab;
                # Hq[Q, 2] += OQ^T @ ab;  SufH = STRI^T @ Hq (strict suffix)
                tmp = rp.tile([P, UT], F32, tag="tmp")
                m_col = rp.tile([P, UT], F32, tag="m_col")
                q_col_i = rp.tile([P, UT], I16, tag="q_col_i")
                q_col = rp.tile([P, UT], F32, tag="q_col")
                s_col = rp.tile([P, UT], F32, tag="s_col")
                f_ps = psA.tile([P, 64], F32, tag="f_ps")
                h_ps = psA.tile([P, 2], F32, tag="h_ps")

                nc.vector.tensor_scalar(
                    tmp[:], rsum[:], float(N - 1), -1.0, OP.subtract, OP.mult
                )
                nc.vector.tensor_tensor(m_col[:], rsum[:], tmp[:], OP.min)
                if debug:
                    nc.sync.dma_start(dbg_m.ap(), m_col[:])
                nc.vector.tensor_scalar(
                    q_col_i[:], m_col[:], 1.0 / 32.0, -15.5 / 32.0,
                    OP.mult, OP.add,
                )
                nc.vector.tensor_copy(q_col[:], q_col_i[:])
                nc.vector.scalar_tensor_tensor(
                    s_col[:], q_col[:], -32.0, m_col[:], OP.mult, OP.add
                )
                hq_ps = psA.tile([P, 2], F32, tag="hq_ps")
                for k in range(UT):
                    oq = work.tile([P, NQ], BF16, tag="oq")
                    nc.vector.tensor_scalar(
                        oq[:], iota_row_f[:], q_col[:, k : k + 1], None,
                        OP.is_equal,
                    )
                    ssab = work.tile([P, 2 * NS], BF16, tag="ssab")
                    nc.vector.tensor_scalar(
                        ssab[:, 0:NS], iotaS[:], s_col[:, k : k + 1],
                        ab[:, k, 0:1], OP.is_le, OP.mult,
                    )
                    nc.vector.tensor_scalar(
                        ssab[:, NS : 2 * NS], iotaS[:], s_col[:, k : k + 1],
                        ab[:, k, 1:2], OP.is_le, OP.mult,
                    )
                    nc.tensor.matmul(
                        f_ps[:], lhsT=oq[:], rhs=ssab[:],
                        start=(k == 0), stop=(k == UT - 1),
                    )
                    nc.tensor.matmul(
                        hq_ps[:], lhsT=oq[:], rhs=ab_bf[:, k, :],
                        start=(k == 0), stop=(k == UT - 1),
                    )
                hq_sb = rp.tile([P, 2], F32, tag="hq_sb")
                nc.vector.tensor_copy(hq_sb[:], hq_ps[:])
                nc.tensor.matmul(
                    h_ps[:], lhsT=stri[:], rhs=hq_sb[:], start=True, stop=True
                )

                # num partial: sum_u pred_u * (2*[r_u < N/2] - 1)
                sgn = rp.tile([P, UT], F32, tag="sgn")
                nc.vector.tensor_scalar(sgn[:], rsum[:], float(NPAIR), None, OP.is_lt)
                nc.vector.tensor_scalar(sgn[:], sgn[:], 2.0, -1.0, OP.mult, OP.add)
                xp = rp.tile([P, UT], F32, tag="xp")
                nc.vector.tensor_tensor(xp[:], sgn[:], pcol_sb, OP.mult)
                xq = rp.tile([P, 1], F32, tag="xq")
                nc.vector.tensor_reduce(
                    xq[:], xp[:], axis=mybir.AxisListType.X, op=OP.add
                )
                np_ps = sm_ps[0:1, 1:2]
                nc.tensor.matmul(
                    np_ps, lhsT=xq[:], rhs=ones_col[:], start=True, stop=True
                )

                fh_in = rp.tile([P, 67], F16, tag="fh_in")
                nc.vector.tensor_copy(fh_in[:, 0:64], f_ps[:])
                nc.vector.tensor_copy(fh_in[:, 64:66], h_ps[:])
                nc.vector.memset(fh_in[:, 66:67], 0.0)
                nc.vector.tensor_copy(fh_in[0:1, 66:67], np_ps)

                # ---- collective 2: ReduceScatter F/SufH/num ----
                QC = P // NCORE  # 16 Q rows per core
                fh_dram = dram.tile([P, 67], F16, tag="fh_dram")
                nc.sync.dma_start(fh_dram[:], fh_in[:])
                fhrs_dram = dram.tile([QC, 67], F16, tag="fhrs_dram")
                if coll2:
                    nc.gpsimd.collective_compute(
                        "ReduceScatter",
                        OP.add,
                        replica_groups=[list(range(NCORE))],
                        ins=[fh_dram[:].opt()],
                        outs=[fhrs_dram[:].opt()],
                    )
                else:
                    nc.sync.dma_start(fhrs_dram[:], fh_dram[0:QC, :])
                fh_sb = rp.tile([QC, 67], F16, tag="fh_sb")
                nc.sync.dma_start(fh_sb[:], fhrs_dram[:])
                if debug:
                    nc.sync.dma_start(dbg_fh.ap(), fh_sb[:])

                # ---- phase 3: denom/ln on this core's 512 windows ----
                suf3 = rp.tile([QC, 3], F32, tag="suf3")
                nc.vector.tensor_copy(suf3[:], fh_sb[:, 64:67])
                at = rp.tile([QC, NS], F32, tag="at")
                nc.vector.tensor_scalar(
                    at[:], fh_sb[:, 0:NS], suf3[:, 0:1], None, OP.add
                )
                bt = rp.tile([QC, NS], F32, tag="bt")
                nc.vector.tensor_scalar(
                    bt[:], fh_sb[:, NS : 2 * NS], suf3[:, 1:2], None, OP.add
                )
                den = rp.tile([QC, NS], F32, tag="den")
                nc.vector.tensor_tensor(den[:], at[:], bt[:], OP.mult)
                nc.vector.tensor_tensor(den[:], den[:], win_sb[0:QC, :], OP.subtract)
                # quantization ties can empty the innermost window; mirror the
                # reference's where(denom <= 0, EPS, denom) guard
                nc.vector.tensor_scalar(den[:], den[:], 1e-8, None, OP.max)
                logd = rp.tile([QC, NS], F32, tag="logd")
                lnacc = rp.tile([QC, 1], F32, tag="lnacc")
                nc.scalar.activation(logd[:], den[:], AF.Ln, accum_out=lnacc[:])
                ln_ps = sm_ps[0:1, 2:3]
                nc.tensor.matmul(
                    ln_ps, lhsT=lnacc[:], rhs=ones_col[0:QC, :],
                    start=True, stop=True,
                )
                out_sb = rp.tile([1, 1], F32, tag="out_sb")
                nc.vector.tensor_tensor(
                    out_sb[:], ln_ps, suf3[0:1, 2:3], OP.subtract
                )
                nc.sync.dma_start(out_part.ap(), out_sb[:])

    nc.compile()
    return nc


def make_in_maps(pred: np.ndarray, target: np.ndarray):
    pred = np.ascontiguousarray(pred, dtype=np.float32).reshape(N)
    target = np.ascontiguousarray(target, dtype=np.float32).reshape(N)
    in_maps = []
    for c in range(NCORE):
        tsl = target[c * US : (c + 1) * US]
        psl = pred[c * US : (c + 1) * US]
        win = np.zeros((P, NS), np.float32)
        rho = np.arange(P // NCORE)
        s_i = np.arange(NS)
        win[: P // NCORE, :] = (
            N - 64.0 * (16 * c + rho)[:, None] - 2.0 * s_i[None, :]
        )
        pk = np.concatenate(
            [tsl.reshape(UT, P).T, psl.reshape(UT, P).T, win], axis=1
        ).astype(np.float32)
        in_maps.append(
            {
                "t_own": np.ascontiguousarray(tsl.reshape(1, US)),
                "tqcol": np.ascontiguousarray(tsl.reshape(UT, P).T),
                "packed": np.ascontiguousarray(pk),
            }
        )
    return in_maps


_CACHE = {}


def _get_module():
    if "nc" not in _CACHE:
        _CACHE["nc"] = build_module(debug=False)
    return _CACHE["nc"]


def kernel(pred: np.ndarray, target: np.ndarray) -> np.ndarray:
    from concourse import bass_utils

    nc = _get_module()
    in_maps = make_in_maps(pred, target)
    res = bass_utils.run_bass_kernel_spmd(nc, in_maps, core_ids=list(range(NCORE)))
    total = np.float32(0.0)
    for c in range(NCORE):
        total = np.float32(total + res.results[c]["out_part"][0, 0])
    return np.asarray(total, dtype=np.float32)
